# revision 37
# baseline (speedup 1.0000x reference)
"""Trainium2 Bass kernel for nn_BertEmbedding_1623497638029.

Per batch row b and token t (T=256 tokens, P=512 subword positions,
H=768), with subword counts lens in {0,1,2}:

    cum  = cumsum(bert_lens[b])
    lo_t = cum[t] - lens[t]        # first subword of token t
    hi_t = cum[t] - 1              # last subword of token t
    out[b,t] = mean(enc[b, lo_t:hi_t+1]) if lens[t] else 0

Implementation: tokens are processed as 128 PAIRS per batch row. One
indirect-DMA gather per batch row fetches a 4-row window per pair,
rows w..w+3 with w = cum[2p] - 2 (the even token's last two rows are
always window rows {0,1}; the odd token's rows are always {2,3}):

    out_even = a0*r0 + b0*r1
    out_odd  = b1*r2 + a1*r3
    a = 0.5*(len>1),  b = (len>0) - a   (per token)

so each output is a 2-term weighted sum with NO position indicators —
4 elementwise ops per batch row instead of 8 (differential timing
showed the elementwise combine, not DMA, was the bottleneck).

The window start is NOT clamped: w = cum[0]-2 = -1 for pair 0 when
lens[0] == 1, but then a0 = 0 so the out-of-range row r0 gets a zero
coefficient.  To keep the DMA index non-negative, enc is uploaded
with ONE junk row prepended ([BL*P+1, H]) and every index shifted +1;
the max index cum[2p]+1+1+boff is exactly BL*P, in bounds.

IO is fp16 (tolerance is 2e-2; fp16 rounding is ~1e-4 on the norm):
the host casts enc to fp16 and upcasts the output, halving HBM
traffic vs f32.

Sharding: pure data parallel - 8 batch rows per NeuronCore, 8 cores,
no cross-core communication.
"""

import numpy as np

import concourse.bacc as bacc
import concourse.bass as bass
import concourse.mybir as mybir
import concourse.tile as tile
from concourse.bass_utils import run_bass_kernel_spmd
from concourse.masks import make_identity

NCORES = 8
BZ, P, T, H = 64, 512, 256, 768
BL = BZ // NCORES  # batch rows per core
NCH = T // 128  # 128-token chunks per batch row

F32 = mybir.dt.float32
F16 = mybir.dt.float16
I32 = mybir.dt.int32
ALU = mybir.AluOpType
AF = mybir.ActivationFunctionType


def _sel_mask(nc, t, base):
    """t[k,p] = 1 iff base + k - 2p == 0."""
    nc.gpsimd.memset(t, 0.0)
    nc.gpsimd.affine_select(
        out=t, in_=t, compare_op=ALU.not_equal, fill=1.0,
        base=base, pattern=[[-2, 128]], channel_multiplier=1,
    )


def _le_mask(nc, t, base):
    """t[k,p] = 1 iff base + k - 2p <= 0."""
    nc.gpsimd.memset(t, 0.0)
    nc.gpsimd.affine_select(
        out=t, in_=t, compare_op=ALU.is_gt, fill=1.0,
        base=base, pattern=[[-2, 128]], channel_multiplier=1,
    )


def _build_nc(acc_bufs=6, res_bufs=4, tmp_bufs=4, repeat=0, asserts=True,
              no_gather=False, no_store=False, no_compute=False,
              cmode="sv", oob_skip=True, gather_rows=4, max_unroll=4,
              store_alt=False):
    nc = bacc.Bacc(
        "TRN2", target_bir_lowering=False, debug=False,
        num_devices=NCORES, enable_asserts=asserts,
    )
    # enc is padded with one junk row at the top so the pair-0 window
    # start cum[0]-2+1 is never negative; junk rows only ever meet zero
    # coefficients.
    enc = nc.dram_tensor("enc", [BL * P + 1, H], F16, kind="ExternalInput").ap()
    lens = nc.dram_tensor("lens", [BL, T], I32, kind="ExternalInput").ap()
    out = nc.dram_tensor("out", [BL, T, H], F16, kind="ExternalOutput").ap()

    with tile.TileContext(nc) as tc:
        with (
            tc.tile_pool(name="const", bufs=1) as cpool,
            tc.tile_pool(name="idx", bufs=1) as ipool,
            tc.tile_pool(name="psum", bufs=2, space="PSUM") as ppool,
            tc.tile_pool(name="acc", bufs=acc_bufs) as apool,
            tc.tile_pool(name="tmp", bufs=tmp_bufs) as tpool,
            tc.tile_pool(name="res", bufs=res_bufs) as rpool,
        ):
            # ---- constants ----
            ident = cpool.tile([128, 128], F32)
            make_identity(nc, ident[:])
            # selection/prefix masks: columns are pairs p, rows chunk-local k
            mcum0 = cpool.tile([128, 128], F32)  # k <= 2p (chunk 0)
            _le_mask(nc, mcum0[:], base=0)
            mcum1 = cpool.tile([128, 128], F32)  # 128+k <= 2p
            _le_mask(nc, mcum1[:], base=128)
            se0 = cpool.tile([128, 128], F32)  # k == 2p (chunk 0)
            _sel_mask(nc, se0[:], base=0)
            se1 = cpool.tile([128, 128], F32)  # 128+k == 2p
            _sel_mask(nc, se1[:], base=128)
            so0 = cpool.tile([128, 128], F32)  # k == 2p+1 (chunk 0)
            _sel_mask(nc, so0[:], base=-1)
            so1 = cpool.tile([128, 128], F32)  # 128+k == 2p+1
            _sel_mask(nc, so1[:], base=127)
            boff_i = cpool.tile([128, BL], I32)  # per-column batch row offset
            nc.gpsimd.iota(boff_i[:], pattern=[[P, BL]], base=0, channel_multiplier=0)
            boff = cpool.tile([128, BL], F32)
            nc.vector.tensor_copy(boff[:], boff_i[:])
            badj = cpool.tile([1, BL], F32)  # 512*n - 2 + 1 per column
            nc.vector.tensor_scalar_add(badj[:], boff[0:1, :], -1.0)

            # ---- load lens, cast, transpose chunks to [token, batch] ----
            lens_i = ipool.tile([BL, T], I32)
            nc.sync.dma_start(out=lens_i[:], in_=lens[:, :])
            lens_f = ipool.tile([BL, T], F32)
            nc.vector.tensor_copy(lens_f[:], lens_i[:])
            lensT = []
            for c in range(NCH):
                ps_t = ppool.tile([128, BL], F32, tag="tr")
                nc.tensor.transpose(
                    out=ps_t[:], in_=lens_f[:, c * 128 : (c + 1) * 128],
                    identity=ident[0:BL, 0:BL],
                )
                lt = ipool.tile([128, BL], F32, tag=f"lensT{c}")
                nc.vector.tensor_copy(lt[:], ps_t[:])
                lensT.append(lt)

            # ---- pair-layout quantities via selection matmuls ----
            def _accum(masks, name, extra=None):
                pt = ppool.tile([128, BL], F32, tag=name)
                nc.tensor.matmul(out=pt[:], lhsT=masks[0][:], rhs=lensT[0][:],
                                 start=True, stop=False)
                nc.tensor.matmul(out=pt[:], lhsT=masks[1][:], rhs=lensT[1][:],
                                 start=False, stop=(extra is None))
                if extra is not None:
                    # rank-1 accumulate: mcum0 row 0 is all ones
                    nc.tensor.matmul(out=pt[:], lhsT=mcum0[0:1, :],
                                     rhs=extra[:], start=False, stop=True)
                return pt

            c0v = _accum((mcum0, mcum1), "c0v", extra=badj)  # cum[2p]-2+boff
            l0v = _accum((se0, se1), "l0v")      # lens[2p]
            l1v = _accum((so0, so1), "l1v")      # lens[2p+1]

            # ---- window start and coefficients (all [128, BL]) ----
            wg = ipool.tile([128, BL], I32)  # cum[2p]-2+1 + boff (>= 0)
            if oob_skip:
                # push fully-padded pairs (l0+l1==0) out of bounds so the
                # gather skips their descriptors entirely
                l1s = ipool.tile([128, BL], F32)
                nc.vector.tensor_copy(l1s[:], l1v[:])
                z = ipool.tile([128, BL], F32)
                nc.vector.tensor_tensor(out=z[:], in0=l0v[:], in1=l1s[:],
                                        op=ALU.add)
                nc.vector.tensor_scalar(out=z[:], in0=z[:], scalar1=0.0,
                                        scalar2=float(10 * BL * P),
                                        op0=ALU.is_equal, op1=ALU.mult)
                nc.vector.tensor_add(wg[:], c0v[:], z[:])
            else:
                nc.vector.tensor_copy(wg[:], c0v[:])

            def ab(lv, tag):
                a = ipool.tile([128, BL], F32, tag=f"a{tag}")
                nc.vector.tensor_scalar(out=a[:], in0=lv[:], scalar1=1.0,
                                        scalar2=0.5, op0=ALU.is_gt, op1=ALU.mult)
                g = ipool.tile([128, BL], F32, tag=f"g{tag}")
                nc.vector.tensor_scalar(out=g[:], in0=lv[:], scalar1=0.0,
                                        scalar2=None, op0=ALU.is_gt)
                b = ipool.tile([128, BL], F32, tag=f"b{tag}")
                nc.vector.tensor_sub(b[:], g[:], a[:])
                return a, b

            a0, b0 = ab(l0v, "0")
            a1, b1 = ab(l1v, "1")

            # ---- main loop ----
            if oob_skip:
                # one-time: make the acc ring NaN-free so skipped (stale)
                # partitions stay finite; zero coefficients do the rest.
                for _ in range(acc_bufs):
                    t = apool.tile([128, gather_rows * H], F16, tag="acc")
                    nc.gpsimd.memset(t[:], 0.0)

            def main_body(_iv=None):
                for b in range(BL):
                    acc = apool.tile([128, gather_rows * H], F16, tag="acc")
                    if not no_gather:
                        nc.gpsimd.indirect_dma_start(
                            out=acc[:], out_offset=None, in_=enc[:, :],
                            in_offset=bass.IndirectOffsetOnAxis(
                                ap=wg[:, b : b + 1], axis=0),
                            bounds_check=(BL * P) if oob_skip else None,
                            oob_is_err=not oob_skip,
                        )
                    else:
                        nc.gpsimd.memset(acc[:, 0:4], 0.0)
                    res = rpool.tile([128, 2 * H], F16, tag="res")
                    if not no_compute:
                        r = [acc[:, k * H : (k + 1) * H] for k in range(4)]
                        # even: a0*r0 + b0*r1 ; odd: b1*r2 + a1*r3
                        te = tpool.tile([128, H], F16, tag="te")
                        to = tpool.tile([128, H], F16, tag="to")
                        if cmode == "sv":
                            nc.scalar.activation(out=te[:], in_=r[0],
                                                 func=AF.Copy,
                                                 scale=a0[:, b : b + 1])
                            nc.scalar.activation(out=to[:], in_=r[2],
                                                 func=AF.Copy,
                                                 scale=b1[:, b : b + 1])
                        elif cmode == "v4":
                            nc.vector.tensor_scalar_mul(
                                out=te[:], in0=r[0], scalar1=a0[:, b : b + 1])
                            nc.vector.tensor_scalar_mul(
                                out=to[:], in0=r[2], scalar1=b1[:, b : b + 1])
                        elif cmode == "pv":
                            nc.gpsimd.tensor_scalar_mul(
                                out=te[:], in0=r[0], scalar1=a0[:, b : b + 1])
                            nc.gpsimd.tensor_scalar_mul(
                                out=to[:], in0=r[2], scalar1=b1[:, b : b + 1])
                        elif cmode == "spv":
                            nc.scalar.activation(out=te[:], in_=r[0],
                                                 func=AF.Copy,
                                                 scale=a0[:, b : b + 1])
                            nc.gpsimd.tensor_scalar_mul(
                                out=to[:], in0=r[2], scalar1=b1[:, b : b + 1])
                        else:
                            raise ValueError(cmode)
                        nc.vector.scalar_tensor_tensor(
                            out=res[:, 0:H], in0=r[1], scalar=b0[:, b : b + 1],
                            in1=te[:], op0=ALU.mult, op1=ALU.add)
                        nc.vector.scalar_tensor_tensor(
                            out=res[:, H : 2 * H], in0=r[3],
                            scalar=a1[:, b : b + 1],
                            in1=to[:], op0=ALU.mult, op1=ALU.add)
                    else:
                        nc.vector.tensor_copy(res[:, 0:4], acc[:, 0:4])
                    if not no_store:
                        dest = out[b : b + 1, :, :].rearrange(
                            "o (tp q) h -> (o tp) q h", q=2
                        )
                        eng = nc.scalar if (store_alt and b % 2) else nc.sync
                        eng.dma_start(
                            out=dest,
                            in_=res[:].rearrange("p (q h) -> p q h", h=H))

            if repeat:
                # timing mode: run the steady-state body `repeat` times
                tc.For_i_unrolled(0, repeat, 1, main_body,
                                  max_unroll=max_unroll)
            else:
                main_body()

    nc.compile()
    return nc


_NC = None


def _get_nc():
    global _NC
    if _NC is None:
        _NC = _build_nc()
    return _NC


def kernel(enc_out, bert_mask, bert_lens):
    del bert_mask  # implied by bert_lens (mask = arange(P) < cumsum(lens)[-1])
    enc_np = np.ascontiguousarray(np.asarray(enc_out, dtype=np.float32))
    lens_np = np.ascontiguousarray(np.asarray(bert_lens, dtype=np.int32))
    assert enc_np.shape == (BZ, P, H) and lens_np.shape == (BZ, T)
    enc_np = enc_np.astype(np.float16)

    nc = _get_nc()
    pad = np.zeros((1, H), dtype=np.float16)
    in_maps = [
        {
            "enc": np.concatenate(
                [pad, enc_np[i * BL : (i + 1) * BL].reshape(BL * P, H)]),
            "lens": lens_np[i * BL : (i + 1) * BL],
        }
        for i in range(NCORES)
    ]
    results = run_bass_kernel_spmd(nc, in_maps, core_ids=list(range(NCORES))).results
    out = np.concatenate([np.asarray(r["out"]) for r in results], axis=0)
    return out.reshape(BZ, T, H).astype(np.float32)
